# revision 6
# baseline (speedup 1.0000x reference)
"""DetContrastiveLoss Trainium2 kernel.

Two SPMD phases over 8 NeuronCores (no ncfw collectives — their entry
barrier + launch skew costs more than the 1MB exchange itself):

  Phase A (per core k): own 128 boxes of batch b=k//2. Compute box pixel
    indices on-device (exact f32 chain matching the reference), gather the
    64-float aligned HBM window holding each (box, channel) scalar via
    dma_gather. The gather is issued as 32 sub-calls of 1024 windows
    round-robined over 4 SWDGE queues (single-queue SWDGE serializes at
    ~28 GB/s: each SDMA ring holds one outstanding 256B read, so
    throughput is rings x 256B / HBM latency; 4 queues x 16 rings
    saturates at ~105 GB/s). Extract the exact element with a one-hot
    mask, L2-normalize rows (1/sqrt(temperature) folded in), transpose on
    PE -> fnT block [256, 128].
  Host: concat blocks and permute columns into atom-group-sorted order
    (6 groups: {dyn,static} x class; sorting is host-side marshalling of
    the exchange buffer, the maxima still happen on device).
  Phase B (per core k): sim block [128, 1024] = own_fnT.T @ fnT via PE;
    per-atom column maxima via tensor_mask_reduce with runtime [lo,hi)
    bounds (no bias tensors, no DRAM bounce); hinge, anchor-masked
    partial sums -> [1, 6] per core.
  Host: assemble the scalar loss from the 8x6 partials + host-side atom
    counts (f32 arithmetic mirroring the reference).
"""

import sys

for _p in ("/opt/trn_rl_repo", "/root/.axon_site/_ro/trn_rl_repo"):
    if _p not in sys.path:
        sys.path.append(_p)

import numpy as np

import concourse.bass as bass
import concourse.bacc as bacc
import concourse.tile as tile
import concourse.mybir as mybir
from concourse import bass_utils
from concourse.masks import make_identity

F32 = mybir.dt.float32
I32 = mybir.dt.int32
I16 = mybir.dt.int16

B, N, C, H, W = 4, 256, 256, 360, 360
HW = H * W            # 129600
CHW = C * HW          # 33177600
M = B * N             # 1024
NCORES = 8
BOX = 128             # boxes per core
TEMPERATURE = 0.1
MARGIN = 0.2
X0 = -59.9
SPAN = 119.8
CH_CHUNK = 16         # channels per gather chunk (int16 index limit)
NCALLS = C // CH_CHUNK
SPLITS = 2            # sub-calls per chunk, round-robined over queues
NQUEUES = 4
ROWS_PER_CH = HW // 64   # 2025 aligned 64-float windows per channel plane
SLOT = 256            # phase B column slot per atom group (counts ~170±13)
MPAD = 6 * SLOT       # 1536 padded sim columns
SQRT_INV_T = float(np.sqrt(np.float32(1.0) / np.float32(TEMPERATURE)))

AX = mybir.AxisListType
ALU = mybir.AluOpType


def _coord_chain(nc, pool, shape, src_ap, w_dim, tag):
    """clip((x - X0)/SPAN * w_dim, 0, w_dim-1) then floor -> float tile.

    Matches jnp: sub, div, mul, clip, trunc. Floor is computed as
    int-cast + cast-back + fix so it is exact under any HW cast rounding.
    """
    t = pool.tile(shape, F32, tag=f"{tag}_t")
    inv_span = float(np.float32(1.0) / np.float32(SPAN))
    nc.vector.tensor_scalar(out=t[:], in0=src_ap, scalar1=float(X0), scalar2=inv_span, op0=ALU.subtract, op1=ALU.mult)
    nc.vector.tensor_scalar(out=t[:], in0=t[:], scalar1=float(w_dim), scalar2=None, op0=ALU.mult)
    nc.vector.tensor_scalar(out=t[:], in0=t[:], scalar1=0.0, scalar2=float(w_dim - 1), op0=ALU.max, op1=ALU.min)
    return _floor(nc, pool, shape, t, tag)


def _floor(nc, pool, shape, t, tag):
    ti = pool.tile(shape, I32, tag=f"{tag}_i")
    nc.vector.tensor_copy(out=ti[:], in_=t[:])
    tb = pool.tile(shape, F32, tag=f"{tag}_b")
    nc.vector.tensor_copy(out=tb[:], in_=ti[:])
    gt = pool.tile(shape, F32, tag=f"{tag}_g")
    nc.vector.tensor_tensor(out=gt[:], in0=tb[:], in1=t[:], op=ALU.is_gt)
    fl = pool.tile(shape, F32, tag=f"{tag}_f")
    nc.vector.tensor_tensor(out=fl[:], in0=tb[:], in1=gt[:], op=ALU.subtract)
    return fl


def build_phase_a():
    nc = bacc.Bacc("TRN2", target_bir_lowering=False, debug=False,
                   num_devices=NCORES, num_swdge_queues=NQUEUES,
                   dynamic_dma_scratch_size=65536)
    spatial = nc.dram_tensor("spatial", [CHW], F32, kind="ExternalInput")
    boxes = nc.dram_tensor("boxes", [BOX, 9], F32, kind="ExternalInput")
    fnt_out = nc.dram_tensor("fnt", [C, BOX], F32, kind="ExternalOutput")

    with tile.TileContext(nc) as tc:
        with tc.tile_pool(name="sb", bufs=1) as pool, \
             tc.tile_pool(name="win", bufs=3) as winp, \
             tc.tile_pool(name="ps", bufs=2, space="PSUM") as psp:
            # ---- load boxes in two layouts ----
            bx = pool.tile([BOX, 9], F32)
            nc.sync.dma_start(out=bx[:], in_=boxes.ap())
            bx16 = pool.tile([16, 8, 9], F32)
            nc.sync.dma_start(
                out=bx16[:],
                in_=boxes.ap().rearrange("(j q) f -> q j f", q=16),
            )

            # ---- per-box (128-partition layout): R, G, o ----
            cx = _coord_chain(nc, pool, [BOX, 1], bx[:, 0:1], W, "cx")
            cy = _coord_chain(nc, pool, [BOX, 1], bx[:, 1:2], H, "cy")
            r = pool.tile([BOX, 1], F32)
            nc.vector.tensor_scalar(out=r[:], in0=cy[:], scalar1=float(W), scalar2=None, op0=ALU.mult)
            nc.vector.tensor_tensor(out=r[:], in0=r[:], in1=cx[:], op=ALU.add)
            g_pre = pool.tile([BOX, 1], F32)
            nc.vector.tensor_scalar(out=g_pre[:], in0=r[:], scalar1=float(1.0 / 64.0), scalar2=None, op0=ALU.mult)
            g = _floor(nc, pool, [BOX, 1], g_pre, "g")
            o = pool.tile([BOX, 1], F32)
            nc.vector.tensor_scalar(out=o[:], in0=g[:], scalar1=float(-64.0), scalar2=None, op0=ALU.mult)
            nc.vector.tensor_tensor(out=o[:], in0=r[:], in1=o[:], op=ALU.add)

            # ---- same chain on the [16, 8] wrapped layout -> G16 ----
            cx16 = _coord_chain(nc, pool, [16, 8, 1], bx16[:, :, 0:1], W, "cx16")
            cy16 = _coord_chain(nc, pool, [16, 8, 1], bx16[:, :, 1:2], H, "cy16")
            r16 = pool.tile([16, 8, 1], F32)
            nc.vector.tensor_scalar(out=r16[:], in0=cy16[:], scalar1=float(W), scalar2=None, op0=ALU.mult)
            nc.vector.tensor_tensor(out=r16[:], in0=r16[:], in1=cx16[:], op=ALU.add)
            g16_pre = pool.tile([16, 8, 1], F32)
            nc.vector.tensor_scalar(out=g16_pre[:], in0=r16[:], scalar1=float(1.0 / 64.0), scalar2=None, op0=ALU.mult)
            g16 = _floor(nc, pool, [16, 8, 1], g16_pre, "g16")

            # idx[q, j, c_rel] = G16[q, j] + 2025*c_rel  (f32, exact)
            iota_c = pool.tile([16, 8, CH_CHUNK], I32)
            nc.gpsimd.iota(iota_c[:], pattern=[[0, 8], [ROWS_PER_CH, CH_CHUNK]], base=0, channel_multiplier=0)
            iota_cf = pool.tile([16, 8, CH_CHUNK], F32)
            nc.vector.tensor_copy(out=iota_cf[:], in_=iota_c[:])
            idx_f = pool.tile([16, 8, CH_CHUNK], F32)
            nc.vector.tensor_tensor(
                out=idx_f[:],
                in0=iota_cf[:],
                in1=g16[:].to_broadcast([16, 8, CH_CHUNK]),
                op=ALU.add,
            )
            # reorder free dims to (c_rel, j) and cast to int16.
            # dma_gather reads the idx AP as [128, num_idxs/16]: each of the
            # 8 gpsimd cores reads its own 16-partition group, expecting a
            # replica of the same index table -> replicate by doubling.
            idx16 = pool.tile([128, CH_CHUNK, 8], I16)
            nc.vector.tensor_copy(out=idx16[:16], in_=idx_f[:].rearrange("q j c -> q c j"))
            for lo, n in ((16, 16), (32, 32), (64, 64)):
                nc.sync.dma_start(out=idx16[lo:lo + n], in_=idx16[:n])

            # ---- one-hot extraction mask [128, CH_CHUNK, 64] ----
            iota_d = pool.tile([BOX, CH_CHUNK, 64], I32)
            nc.gpsimd.iota(iota_d[:], pattern=[[0, CH_CHUNK], [1, 64]], base=0, channel_multiplier=0)
            iota_df = pool.tile([BOX, CH_CHUNK, 64], F32)
            nc.vector.tensor_copy(out=iota_df[:], in_=iota_d[:])
            mask = pool.tile([BOX, CH_CHUNK, 64], F32)
            nc.vector.tensor_scalar(out=mask[:], in0=iota_df[:], scalar1=o[:], scalar2=None, op0=ALU.is_equal)

            # ---- gather + extract ----
            feats = pool.tile([BOX, C], F32)
            sub = CH_CHUNK // SPLITS
            qn = 0
            for h in range(NCALLS):
                win = winp.tile([128, CH_CHUNK, 64], F32, tag="win")
                src = spatial.ap()[h * CH_CHUNK * HW:].rearrange("(r e) -> r e", e=64)[: CH_CHUNK * ROWS_PER_CH, :]
                for s in range(SPLITS):
                    nc.gpsimd.dma_gather(
                        out_ap=win[:, s * sub:(s + 1) * sub, :],
                        in_ap=src,
                        idxs_ap=idx16[:, s * sub:(s + 1) * sub, :].rearrange("q c j -> q (c j)"),
                        num_idxs=sub * 128,
                        num_idxs_reg=sub * 128,
                        elem_size=64,
                        single_packet=False,
                        queue_num=qn % NQUEUES,
                    )
                    qn += 1
                prod = winp.tile([128, CH_CHUNK, 64], F32, tag="prod")
                nc.vector.tensor_tensor(out=prod[:], in0=win[:], in1=mask[:], op=ALU.mult)
                nc.vector.tensor_reduce(
                    out=feats[:, h * CH_CHUNK:(h + 1) * CH_CHUNK],
                    in_=prod[:], op=ALU.add, axis=AX.X,
                )

            # ---- normalize rows; fold 1/sqrt(T) ----
            sq = pool.tile([BOX, C], F32)
            nc.vector.tensor_tensor(out=sq[:], in0=feats[:], in1=feats[:], op=ALU.mult)
            ssq = pool.tile([BOX, 1], F32)
            nc.vector.tensor_reduce(out=ssq[:], in_=sq[:], op=ALU.add, axis=AX.X)
            nc.vector.tensor_scalar(out=ssq[:], in0=ssq[:], scalar1=1e-24, scalar2=None, op0=ALU.max)
            rt = pool.tile([BOX, 1], F32)
            nc.vector.reciprocal(out=rt[:], in_=ssq[:])          # 1/ssq
            nc.scalar.activation(rt[:], rt[:], mybir.ActivationFunctionType.Sqrt)  # 1/norm
            # one Newton step on r ~= rsqrt(ssq): r' = r*(1.5 - 0.5*ssq*r^2)
            r2 = pool.tile([BOX, 1], F32)
            nc.vector.tensor_tensor(out=r2[:], in0=rt[:], in1=rt[:], op=ALU.mult)
            nc.vector.tensor_tensor(out=r2[:], in0=r2[:], in1=ssq[:], op=ALU.mult)
            nc.vector.tensor_scalar(out=r2[:], in0=r2[:], scalar1=-0.5, scalar2=1.5, op0=ALU.mult, op1=ALU.add)
            nc.vector.tensor_tensor(out=rt[:], in0=rt[:], in1=r2[:], op=ALU.mult)
            nc.vector.tensor_scalar(out=rt[:], in0=rt[:], scalar1=SQRT_INV_T, scalar2=None, op0=ALU.mult)
            fn = pool.tile([BOX, C], F32)
            nc.vector.tensor_scalar(out=fn[:], in0=feats[:], scalar1=rt[:], scalar2=None, op0=ALU.mult)

            # ---- transpose [128, 256] -> [256, 128] via PE ----
            ident = pool.tile([128, 128], F32)
            make_identity(nc, ident[:])
            fnt_sb = pool.tile([128, 2, 128], F32)
            for hh in range(2):
                pst = psp.tile([128, 128], F32, tag="pst")
                nc.tensor.transpose(out=pst[:], in_=fn[:, hh * 128:(hh + 1) * 128], identity=ident[:])
                nc.vector.tensor_copy(out=fnt_sb[:, hh, :], in_=pst[:])
            nc.sync.dma_start(
                out=fnt_out.ap().rearrange("(h c) b -> c h b", h=2),
                in_=fnt_sb[:],
            )
    nc.compile()
    return nc


def build_phase_b():
    nc = bacc.Bacc("TRN2", target_bir_lowering=False, debug=False, num_devices=NCORES)
    fnt_pad = nc.dram_tensor("fnt_pad", [C, MPAD], F32, kind="ExternalInput")
    ext = nc.dram_tensor("ext", [1, MPAD], F32, kind="ExternalInput")
    own_fnt = nc.dram_tensor("own_fnt", [C, BOX], F32, kind="ExternalInput")
    ownb = nc.dram_tensor("ownb", [BOX, 9], F32, kind="ExternalInput")
    out = nc.dram_tensor("out", [1, 8], F32, kind="ExternalOutput")

    with tile.TileContext(nc) as tc:
        with tc.tile_pool(name="sb", bufs=1) as pool, \
             tc.tile_pool(name="ps", bufs=2, space="PSUM") as psp, \
             tc.tile_pool(name="ps1", bufs=1, space="PSUM") as psp1:
            # ---- load fnT (channel-major), columns slotted by atom group:
            # group a's columns sit at [a*SLOT, a*SLOT+count_a), the rest of
            # the slot is zero-vector padding. The ext row adds -1e9 to every
            # pad column via an extra 1-partition contraction chunk, so slot
            # maxima equal the reference's NEG_FILL-masked maxima exactly.
            rhs = pool.tile([128, 2, MPAD], F32)
            nc.sync.dma_start(out=rhs[:], in_=fnt_pad.ap().rearrange("(h c) j -> c h j", h=2))
            ext_sb = pool.tile([1, MPAD], F32)
            nc.sync.dma_start(out=ext_sb[:], in_=ext.ap())
            lhs = pool.tile([128, 2, BOX], F32)
            nc.sync.dma_start(out=lhs[:], in_=own_fnt.ap().rearrange("(h c) b -> c h b", h=2))
            ones_row = pool.tile([1, BOX], F32)
            nc.vector.memset(ones_row[:], 1.0)

            # ---- sim block [128, MPAD] in PSUM (3 banks) ----
            sim = psp1.tile([128, MPAD], F32)
            for nb in range(MPAD // 512):
                cols = slice(nb * 512, (nb + 1) * 512)
                for hh in range(2):
                    nc.tensor.matmul(
                        out=sim[:, cols],
                        lhsT=lhs[:, hh, :],
                        rhs=rhs[:, hh, cols],
                        start=(hh == 0),
                        stop=False,
                    )
                nc.tensor.matmul(
                    out=sim[:, cols],
                    lhsT=ones_row[:],
                    rhs=ext_sb[:, cols],
                    start=False,
                    stop=True,
                )

            # ---- per-atom slot maxima (compile-time slices) ----
            amax = pool.tile([128, 6], F32)
            for a in range(6):
                nc.vector.tensor_reduce(
                    out=amax[:, a:a + 1],
                    in_=sim[:, a * SLOT:(a + 1) * SLOT],
                    op=ALU.max, axis=AX.X,
                )

            # ---- own-box anchor atoms [128, 6] ----
            ob = pool.tile([BOX, 9], F32)
            nc.sync.dma_start(out=ob[:], in_=ownb.ap())
            oflag = ob[:, 7:8]
            ocls = ob[:, 8:9]
            ostat = pool.tile([BOX, 1], F32)
            nc.vector.tensor_scalar(out=ostat[:], in0=oflag, scalar1=1.0, scalar2=-1.0, op0=ALU.subtract, op1=ALU.mult)
            oatom = pool.tile([BOX, 6], F32)
            for c in range(3):
                e = pool.tile([BOX, 1], F32, tag="oec")
                nc.vector.tensor_scalar(out=e[:], in0=ocls, scalar1=float(c), scalar2=None, op0=ALU.is_equal)
                nc.vector.tensor_tensor(out=oatom[:, c:c + 1], in0=e[:], in1=oflag, op=ALU.mult)
                nc.vector.tensor_tensor(out=oatom[:, 3 + c:4 + c], in0=e[:], in1=ostat[:], op=ALU.mult)

            # ---- hinge per group, anchor-masked ----
            rhs6 = pool.tile([128, 6], F32)
            for g in range(6):
                s_c = 0 if g >= 3 else 1          # opposite-state block
                c = g % 3
                a_pos = s_c * 3 + c
                n1 = s_c * 3 + (c + 1) % 3
                n2 = s_c * 3 + (c + 2) % 3
                mn = pool.tile([BOX, 1], F32, tag="mn")
                nc.vector.tensor_tensor(out=mn[:], in0=amax[:, n1:n1 + 1], in1=amax[:, n2:n2 + 1], op=ALU.max)
                nc.vector.tensor_tensor(out=mn[:], in0=mn[:], in1=amax[:, a_pos:a_pos + 1], op=ALU.subtract)
                nc.vector.tensor_scalar(out=mn[:], in0=mn[:], scalar1=float(MARGIN), scalar2=0.0, op0=ALU.add, op1=ALU.max)
                nc.vector.tensor_tensor(out=rhs6[:, g:g + 1], in0=mn[:], in1=oatom[:, g:g + 1], op=ALU.mult)

            ones = pool.tile([128, 1], F32)
            nc.vector.memset(ones[:], 1.0)
            psum_out = psp.tile([1, 6], F32, tag="po")
            nc.tensor.matmul(out=psum_out[:], lhsT=ones[:], rhs=rhs6[:], start=True, stop=True)
            osb = pool.tile([1, 8], F32)
            nc.vector.memset(osb[:], 0.0)
            nc.vector.tensor_copy(out=osb[:, 0:6], in_=psum_out[:])
            nc.sync.dma_start(out=out.ap(), in_=osb[:])
    nc.compile()
    return nc


_CACHE = {}


def _get_kernels():
    if "a" not in _CACHE:
        _CACHE["a"] = build_phase_a()
        _CACHE["b"] = build_phase_b()
    return _CACHE["a"], _CACHE["b"]


def _atom_key(boxes_flat: np.ndarray) -> np.ndarray:
    """Atom id per box: 0-2 dynamic class 0-2, 3-5 static class 0-2."""
    dyn = boxes_flat[:, 7] != 0
    cls = boxes_flat[:, 8].astype(np.int32)
    return np.where(dyn, cls, 3 + cls)


def _slot_columns(fnt_all: np.ndarray, key: np.ndarray, counts: np.ndarray):
    """Place each atom group's fnT columns at its fixed SLOT; pad columns are
    zero vectors poisoned to -1e9 by the ext row (NEG_FILL semantics)."""
    assert counts.max() <= SLOT, f"atom group overflow: {counts}"
    fnt_pad = np.zeros((C, MPAD), dtype=np.float32)
    ext = np.full((1, MPAD), -1.0e9, dtype=np.float32)
    for a in range(6):
        cols = np.nonzero(key == a)[0]
        lo = a * SLOT
        fnt_pad[:, lo:lo + len(cols)] = fnt_all[:, cols]
        ext[0, lo:lo + len(cols)] = 0.0
    return fnt_pad, ext


def kernel(spatial_features_2d: np.ndarray, gt_boxes: np.ndarray) -> np.ndarray:
    nca, ncb = _get_kernels()
    spatial = np.ascontiguousarray(spatial_features_2d, dtype=np.float32)
    boxes = np.ascontiguousarray(gt_boxes, dtype=np.float32)

    # ---- phase A: gather + normalize + transpose, data-parallel over boxes ----
    in_a = []
    for k in range(NCORES):
        b = k // 2
        n0 = (k % 2) * BOX
        in_a.append({
            "spatial": spatial[b].reshape(-1),
            "boxes": boxes[b, n0:n0 + BOX, :],
        })
    res_a = bass_utils.run_bass_kernel_spmd(nca, in_a, core_ids=list(range(NCORES)))
    blocks = [res_a.results[k]["fnt"] for k in range(NCORES)]       # each [C, BOX]
    fnt_all = np.concatenate(blocks, axis=1)                        # [C, M]

    # ---- host exchange: slot columns by atom group ----
    allb = boxes.reshape(M, 9)
    key = _atom_key(allb)
    counts = np.bincount(key, minlength=6).astype(np.int64)         # [6]
    fnt_pad, ext = _slot_columns(fnt_all, key, counts)

    # ---- phase B: sim block + group maxima + partials ----
    in_b = []
    for k in range(NCORES):
        in_b.append({
            "fnt_pad": fnt_pad,
            "ext": ext,
            "own_fnt": np.ascontiguousarray(blocks[k]),
            "ownb": allb[k * BOX:(k + 1) * BOX, :],
        })
    res_b = bass_utils.run_bass_kernel_spmd(ncb, in_b, core_ids=list(range(NCORES)))
    parts = np.stack([res_b.results[k]["out"][0] for k in range(NCORES)])  # [8, 8]

    # ---- host: assemble the scalar loss (f32, mirrors the reference) ----
    f32 = np.float32
    psums = parts[:, 0:6].astype(np.float32).sum(axis=0, dtype=np.float32)  # [6]
    cnt_f = counts.astype(np.float32)
    total = f32(0.0)
    cnt = f32(0.0)
    for g in range(6):
        n_a = cnt_f[g]
        a_pos = (g + 3) % 6
        s_c = 0 if g >= 3 else 1
        c = g % 3
        n_pos = cnt_f[a_pos]
        n_neg = f32(cnt_f[s_c * 3 + (c + 1) % 3] + cnt_f[s_c * 3 + (c + 2) % 3])
        if (n_a > 0) and (n_pos > 0) and (n_neg > 0):
            total = f32(total + f32(psums[g] / max(n_a, f32(1.0))))
            cnt = f32(cnt + 1.0)
    loss = f32(total / max(cnt, f32(1.0))) if cnt > 0 else f32(0.0)
    return np.asarray(loss, dtype=np.float32)
